# revision 1
# baseline (speedup 1.0000x reference)
"""Trainium2 Bass kernel for AdvancedKANLayer.

Math (per reference):
  xn    = LayerNorm(x) * ln_w + ln_b           (eps=1e-5)
  base  = silu(xn) @ base_weight.T             [B,S,O]
  t     = tanh(xn)
  basis = cos(pi*k*t), k=1..8
  spl   = einsum('bsig,oig->bso', basis, spline_weight)
  out   = base + spl

Strategy: data-parallel over batch (8 cores, one batch entry each, no
collectives).  Per core the whole thing is one K=18432 GEMM:
  out[o, t] = sum_k W_all[k, o] * panel[k, t]
where panel rows are [silu(xn); cos(1*pi*t); ...; cos(8*pi*t)] per
I-chunk, generated on-chip.  cos(k*pi*t) is built from
c1 = cos(pi*t) = 1 - 2*sin(pi*t/2)^2 via Chebyshev product
identities on the VectorEngine (ScalarE Sin is only valid on [-pi,pi]).
Weights are pre-transposed/pre-tiled on the host, cast to bf16; matmul
runs bf16 with f32 PSUM accumulation.

K-step order is ic-major: step s = ic*9 + m (m=0 silu, m=1..8 cos_m),
so the matmul consumes panel tiles in exactly the order generation
produces them.
"""

import math

import numpy as np
import ml_dtypes

import concourse.bass as bass
import concourse.mybir as mybir
import concourse.tile as tile
from concourse import bacc
from concourse import masks
from concourse.bass import ds, ts
from concourse.bass_utils import run_bass_kernel_spmd

F32 = mybir.dt.float32
BF16 = mybir.dt.bfloat16
AF = mybir.ActivationFunctionType
ALU = mybir.AluOpType

EPS = 1e-5

# geometry (full problem, per core)
B = 8
T = 2048          # tokens per core (= S, one batch entry per core)
I = 2048          # input dim
O = 2048          # output dim
G = 8             # cos harmonics
TCH = 512         # token chunk (matmul N)
NCH = T // TCH    # 4
NIC = I // 128    # 16 I-chunks
NM = G + 1        # 9 panel row-groups per ic (silu + 8 cos)
NK = NIC * NM     # 144 k-steps of 128
KG = 4            # k-steps per weight DMA group
NG = NK // KG     # 18
NOT = O // 128    # 16 o-tiles


def build_nc(nT=T, nI=I, nO=O, tch=TCH):
    nch = nT // tch
    nic = nI // 128
    nk = nic * NM
    n_ot = nO // 128
    ntt = tch // 128          # token-tiles per chunk
    kg = KG
    while nk % kg != 0:
        kg //= 2
    ng = nk // kg

    n_race = min(4, n_ot - 1) if n_ot > 1 else 1

    nc = bacc.Bacc("TRN2", target_bir_lowering=False, debug=False)
    x_ext = nc.declare_dram_parameter("x", [nT, nI], F32, isOutput=False)
    lnw_ext = nc.declare_dram_parameter("lnw", [nI], F32, isOutput=False)
    lnb_ext = nc.declare_dram_parameter("lnb", [nI], F32, isOutput=False)
    wt_ext = nc.declare_dram_parameter("wt", [n_ot, ng, 128, kg, 128], BF16, isOutput=False)
    out_ext = nc.declare_dram_parameter("out", [nO, nT], F32, isOutput=True)

    with tile.TileContext(nc) as tc:
        with (
            tc.tile_pool(name="consts", bufs=1) as consts,
            tc.tile_pool(name="xp", bufs=4) as xpool,
            tc.tile_pool(name="statp", bufs=2) as statp,
            tc.tile_pool(name="genp", bufs=2) as genp,
            tc.tile_pool(name="ladp", bufs=1) as ladp,
            tc.tile_pool(name="panelp", bufs=1) as panelp,
            tc.tile_pool(name="wp", bufs=6) as wp,
            tc.tile_pool(name="stgp", bufs=2) as stgp,
            tc.tile_pool(name="tpps", bufs=2, space="PSUM") as tpps,
            tc.tile_pool(name="mmps", bufs=5, space="PSUM") as mmps,
        ):
            identity = consts.tile([128, 128], F32)
            masks.make_identity(nc, identity[:])
            lnw_sb = consts.tile([128, nic], F32)
            nc.sync.dma_start(lnw_sb[:], lnw_ext.rearrange("(f p) -> p f", p=128))
            lnb_sb = consts.tile([128, nic], F32)
            nc.sync.dma_start(lnb_sb[:], lnb_ext.rearrange("(f p) -> p f", p=128))
            eps_sb = consts.tile([128, 1], F32)
            nc.gpsimd.memset(eps_sb[:], EPS)

            state = {}
            tpm = {}

            def preamble(c):
                """x DMA + LN stats + in-place normalize for chunk c.
                Stats/normalize are per token-tile so the first tile is
                ready after one x DMA, not four."""
                xnts = []
                for j in range(ntt):
                    xt = xpool.tile([128, nI], F32, tag="xt")
                    nc.sync.dma_start(xt[:], x_ext[ds((c * ntt + j) * 128, 128), :])
                    bn6 = statp.tile([128, 4, 6], F32, tag="bn6")
                    for q in range(4):
                        nc.vector.bn_stats(
                            bn6[:, q, :], xt[:, ds(q * (nI // 4), nI // 4)]
                        )
                    stats = statp.tile([128, 2], F32, tag="stats")
                    nc.vector.bn_aggr(stats[:], bn6[:])
                    std = statp.tile([128, 1], F32, tag="std")
                    nc.scalar.activation(
                        std[:], stats[:, 1:2], AF.Sqrt, bias=eps_sb[:]
                    )
                    istd = statp.tile([128, 1], F32, tag="istd")
                    nc.vector.reciprocal(istd[:], std[:])
                    nmi = statp.tile([128, 1], F32, tag="nmi")
                    nc.vector.scalar_tensor_tensor(
                        nmi[:], stats[:, 0:1], -1.0, istd[:], ALU.mult, ALU.mult
                    )
                    # normalize in place: xn = (x - mu) * istd
                    nc.scalar.activation(
                        xt[:], xt[:], AF.Identity, bias=nmi[:], scale=istd[:],
                    )
                    xnts.append(xt)
                state[c] = xnts

            def transpose_ic(c, ic):
                """PE-transpose I-chunk ic of chunk c into a PSUM tile."""
                xnts = state[c]
                tp = tpps.tile([128, tch], F32, tag="tp", name=f"tp_{c}_{ic}")
                for j in range(ntt):
                    nc.tensor.transpose(
                        tp[:, ts(j, 128)], xnts[j][:, ts(ic, 128)], identity[:]
                    )
                tpm[(c, ic)] = tp
                return tp

            def gen_chunk(c):
                """tanh/silu + cheb ladder for every I-chunk of chunk c.
                o-tile 0's matmul groups are emitted interleaved so the
                TensorE does real GEMM work (and stays HAM-warm) while the
                panel is being generated."""
                ptiles = [None] * nk
                pss = [
                    mmps.tile([128, tch], F32, tag="ps", name=f"ps{r}_{c}")
                    for r in range(n_race)
                ]
                g_next = 0

                def race_mm(g_hi):
                    # n_race o-tiles race the panel: denser PE bursts keep
                    # the HAM clock-gate warm, and their passes land in
                    # otherwise ACT/DVE-bound window time
                    nonlocal g_next
                    for g in range(g_next, g_hi):
                        for r in range(n_race):
                            wg = wp.tile([128, kg, 128], BF16, tag="wg",
                                         name=f"wg{r}_{c}_{g}")
                            nc.sync.dma_start(wg[:], wt_ext[r, g])
                            for k8 in range(kg):
                                sidx = g * kg + k8
                                nc.tensor.matmul(
                                    pss[r][:], wg[:, k8, :], ptiles[sidx][:],
                                    start=(sidx == 0), stop=(sidx == nk - 1),
                                )
                    g_next = g_hi

                for ic in range(nic):
                    tp = tpm.pop((c, ic), None)
                    if tp is None:
                        tp = transpose_ic(c, ic)
                    lw = lnw_sb[:, ic : ic + 1]
                    lb = lnb_sb[:, ic : ic + 1]

                    def pt(m):
                        s = ic * NM + m
                        t_ = panelp.tile(
                            [128, tch], BF16, tag=f"p{s:03d}", name=f"panel_{c}_{s:03d}"
                        )
                        ptiles[s] = t_
                        return t_

                    th = genp.tile([128, tch], F32, tag="th")
                    nc.scalar.activation(th[:], tp[:], AF.Tanh, bias=lb, scale=lw)

                    p0 = pt(0)
                    nc.scalar.activation(p0[:], tp[:], AF.Silu, bias=lb, scale=lw)
                    sh = genp.tile([128, tch], F32, tag="sh")
                    nc.scalar.activation(sh[:], th[:], AF.Sin, scale=math.pi / 2)


                    def lad(tag):
                        return ladp.tile(
                            [128, tch], F32, tag=tag, name=f"lad_{tag}_{c}_{ic}"
                        )

                    def stt(out, a, s, b):
                        nc.vector.scalar_tensor_tensor(
                            out[:], a[:], s, b[:], ALU.mult, ALU.mult
                        )

                    # c1 = 1 - 2*sh^2
                    u = lad("u")
                    stt(u, sh, -2.0, sh)
                    c1 = lad("c1")
                    nc.vector.tensor_scalar_add(c1[:], u[:], 1.0)
                    # squares on ScalarE to offload DVE
                    sq1 = lad("sq")
                    nc.scalar.square(sq1[:], c1[:])
                    c2 = lad("c2")
                    nc.vector.tensor_scalar(c2[:], sq1[:], 2.0, -1.0, ALU.mult, ALU.add)
                    # c3 = 2*c1*c2 - c1
                    u3 = lad("u")
                    stt(u3, c2, 2.0, c1)
                    c3 = lad("c3")
                    nc.vector.tensor_sub(c3[:], u3[:], c1[:])

                    sq2 = lad("sq")
                    nc.scalar.square(sq2[:], c2[:])
                    c4 = lad("c4")
                    nc.vector.tensor_scalar(c4[:], sq2[:], 2.0, -1.0, ALU.mult, ALU.add)
                    # exports for m=1..4
                    nc.scalar.copy(pt(1)[:], c1[:])
                    nc.scalar.copy(pt(2)[:], c2[:])
                    nc.scalar.copy(pt(3)[:], c3[:])
                    nc.vector.tensor_copy(pt(4)[:], c4[:])
                    # leaves m=5..8 straight to panel (bf16)
                    u5 = lad("u")
                    stt(u5, c3, 2.0, c2)
                    p5 = pt(5)
                    nc.vector.tensor_sub(p5[:], u5[:], c1[:])
                    sq3 = lad("sq")
                    nc.scalar.square(sq3[:], c3[:])
                    nc.vector.tensor_scalar(
                        pt(6)[:], sq3[:], 2.0, -1.0, ALU.mult, ALU.add
                    )
                    u7 = lad("u")
                    stt(u7, c4, 2.0, c3)
                    nc.vector.tensor_sub(pt(7)[:], u7[:], c1[:])
                    sq4 = lad("sq")
                    nc.scalar.square(sq4[:], c4[:])
                    p8 = pt(8)
                    nc.vector.tensor_scalar(
                        p8[:], sq4[:], 2.0, -1.0, ALU.mult, ALU.add
                    )
                    race_mm((NM * (ic + 1)) // kg)
                race_mm(ng)
                for r in range(n_race):
                    stg = stgp.tile([128, tch], F32, tag="stg",
                                    name=f"stg{r}_{c}")
                    nc.vector.tensor_copy(stg[:], pss[r][:])
                    nc.sync.dma_start(
                        out_ext[ds(r * 128, 128), ds(c * tch, tch)], stg[:]
                    )
                return ptiles

            def mm_chunk(c, ptiles, nxt=None):
                for ot in range(n_race, n_ot):
                    ps = mmps.tile([128, tch], F32, tag="ps")
                    for g in range(ng):
                        if nxt is not None and ot == n_ot - 1:
                            if g == ng - 4:
                                transpose_ic(nxt, 0)
                            elif g == ng - 2:
                                transpose_ic(nxt, 1)
                        wg = wp.tile([128, kg, 128], BF16, tag="wg")
                        nc.sync.dma_start(wg[:], wt_ext[ot, g])
                        for k8 in range(kg):
                            s = g * kg + k8
                            nc.tensor.matmul(
                                ps[:],
                                wg[:, k8, :],
                                ptiles[s][:],
                                start=(s == 0),
                                stop=(s == nk - 1),
                            )
                    stg = stgp.tile([128, tch], F32, tag="stg")
                    nc.vector.tensor_copy(stg[:], ps[:])
                    nc.sync.dma_start(
                        out_ext[ds(ot * 128, 128), ds(c * tch, tch)], stg[:]
                    )

            preamble(0)
            for c in range(nch):
                ptiles = gen_chunk(c)
                if c + 1 < nch:
                    preamble(c + 1)
                mm_chunk(c, ptiles, nxt=(c + 1) if c + 1 < nch else None)

    nc.compile()
    return nc


def prep_weights(base_weight, spline_weight, nO=O, nI=I):
    """Host-side: build bf16 W_all in ic-major k-step order, pre-tiled
    for contiguous [128, kg, 128] DMAs: wt[ot, grp, k_in, ks, o_in]."""
    nic = nI // 128
    nk = nic * NM
    n_ot = nO // 128
    kg = KG
    while nk % kg != 0:
        kg //= 2
    ng = nk // kg
    w = np.empty((NM, nI, nO), np.float32)
    w[0] = base_weight.T                      # [i, o]
    for g in range(G):
        w[1 + g] = spline_weight[:, :, g].T   # [i, o]
    # m-major [9, nic, 128, nO] -> ic-major [nic, 9, 128, nO] -> [nk*128, nO]
    w = w.reshape(NM, nic, 128, nO).transpose(1, 0, 2, 3).reshape(nk * 128, nO)
    w = w.reshape(ng, kg, 128, n_ot, 128).transpose(3, 0, 2, 1, 4)
    return np.ascontiguousarray(w.astype(ml_dtypes.bfloat16))


_NC_CACHE = {}


def _get_nc():
    if "nc" not in _NC_CACHE:
        _NC_CACHE["nc"] = build_nc()
    return _NC_CACHE["nc"]


def kernel(x, ln_weight, ln_bias, base_weight, spline_weight):
    x = np.asarray(x, np.float32)
    ln_weight = np.asarray(ln_weight, np.float32)
    ln_bias = np.asarray(ln_bias, np.float32)
    wt = prep_weights(np.asarray(base_weight, np.float32),
                      np.asarray(spline_weight, np.float32))
    nc = _get_nc()
    in_maps = [
        {
            "x": np.ascontiguousarray(x[b]),
            "lnw": ln_weight,
            "lnb": ln_bias,
            "wt": wt,
        }
        for b in range(B)
    ]
    res = run_bass_kernel_spmd(nc, in_maps, core_ids=list(range(B)))
    out = np.stack([res.results[b]["out"].T for b in range(B)])
    return np.ascontiguousarray(out.astype(np.float32))



# revision 6
# speedup vs baseline: 1.0221x; 1.0221x over previous
"""Trainium2 Bass kernel for AdvancedKANLayer (v2).

Math (per reference):
  xn    = LayerNorm(x) * ln_w + ln_b           (eps=1e-5)
  base  = silu(xn) @ base_weight.T             [B,S,O]
  t     = tanh(xn)
  basis = cos(pi*k*t), k=1..8
  spl   = einsum('bsig,oig->bso', basis, spline_weight)
  out   = base + spl

Strategy: data-parallel over batch (8 cores, one batch entry each, no
collectives).  Per core the whole thing is one K=18432 GEMM:
  out[o, t] = sum_k W_all[k, o] * panel[k, t]
where panel rows are [silu(xn); cos(1*pi*t); ...; cos(8*pi*t)] per
I-chunk, generated on-chip via a Chebyshev ladder from
c1 = cos(pi*t) = 1 - 2*sin(pi*t/2)^2 (ScalarE Sin valid on [-pi,pi]).

v2 changes vs v1:
 - x arrives HOST-TRANSPOSED as xt [I, T]: panel generation reads
   i-major tiles straight from DRAM; the 256 PE transposes are gone.
 - LN stats per token via fp32 ones-matmuls on TensorE (column sums of
   x and x^2 accumulated over the 16 i-blocks into one PSUM bank at
   partitions 0/32), tiny row math on DVE, then one gpsimd
   partition_broadcast of [istd | -mu*istd] rows.  Normalization is two
   DVE tensor_tensor ops with free-dim-broadcast APs.
 - k-step order interleaves i-block PAIRS (s = pair*18 + 2m + sub) so
   tanh/silu/sin run at [128,1024] (half the ACT dispatch overhead);
   panel tiles are [128,1024] pair-tiles, matmuls consume 512-halves.
 - KG=8 weight groups (2KB DMA lines, half the issue count), wt bf16.
 - Output written bf16 (host upcasts): half the out-DMA bytes.
 - n_race=5 o-tiles race panel generation; ladder ops split between
   ACT and DVE to balance both near ~120us/chunk.
"""

import math

import numpy as np
import ml_dtypes

import concourse.bass as bass
import concourse.mybir as mybir
import concourse.tile as tile
from concourse import bacc
from concourse.bass import ds, ts
from concourse.bass_utils import run_bass_kernel_spmd

F32 = mybir.dt.float32
BF16 = mybir.dt.bfloat16
AF = mybir.ActivationFunctionType
ALU = mybir.AluOpType

EPS = 1e-5

# geometry (full problem, per core)
B = 8
T = 2048          # tokens per core (= S, one batch entry per core)
I = 2048          # input dim
O = 2048          # output dim
G = 8             # cos harmonics
TCH = 512         # token chunk (matmul N)
NCH = T // TCH    # 4
NIC = I // 128    # 16 I-chunks
NP = NIC // 2     # 8 pairs
NM = G + 1        # 9 panel row-groups per ic (silu + 8 cos)
NK = NIC * NM     # 144 k-steps of 128
KG = 8            # k-steps per weight DMA group
NG = NK // KG     # 18
NOT = O // 128    # 16 o-tiles
N_RACE = 5        # o-tiles racing panel generation


def build_nc(affine=False):
    nc = bacc.Bacc("TRN2", target_bir_lowering=False, debug=False)
    xt_ext = nc.declare_dram_parameter("xt", [I, T], F32, isOutput=False)
    lnw_ext = nc.declare_dram_parameter("lnw", [I], F32, isOutput=False)
    lnb_ext = nc.declare_dram_parameter("lnb", [I], F32, isOutput=False)
    wt_ext = nc.declare_dram_parameter("wt", [NOT, NG, 128, KG, 128], BF16, isOutput=False)
    out_ext = nc.declare_dram_parameter("out", [O, T], BF16, isOutput=True)

    with tile.TileContext(nc) as tc:
        with (
            tc.tile_pool(name="consts", bufs=1) as consts,
            tc.tile_pool(name="xsp", bufs=2) as xsp,       # stats x stream [128,512] f32
            tc.tile_pool(name="sqp", bufs=2) as sqp,       # squares [128,512] f32
            tc.tile_pool(name="xgp", bufs=3) as xgp,       # gen pair tiles + tanh scratch [128,1024] f32
            tc.tile_pool(name="shp", bufs=1) as shp,       # sin tile [128,1024] f32
            tc.tile_pool(name="scrp", bufs=3) as scrp,     # ladder scratch [128,512] f32
            tc.tile_pool(name="ladp", bufs=1) as ladp,     # c1..c4 [128,512] f32
            tc.tile_pool(name="rowp", bufs=1) as rowp,     # stat rows [1,*] f32
            tc.tile_pool(name="bcp", bufs=1) as bcp,       # broadcast [128,1024] f32
            tc.tile_pool(name="panelp", bufs=1) as panelp, # 72 pair-tiles [128,1024] bf16
            tc.tile_pool(name="wp", bufs=4) as wp,         # weights [128,KG,128] bf16
            tc.tile_pool(name="stgp", bufs=2) as stgp,     # out staging [128,512] bf16
            tc.tile_pool(name="statps", bufs=1, space="PSUM") as statps,
            tc.tile_pool(name="mmps", bufs=7, space="PSUM") as mmps,
        ):
            eps_sb = consts.tile([1, 1], F32)
            nc.gpsimd.memset(eps_sb[:], EPS)
            ones_sb = consts.tile([128, 1], F32)
            nc.gpsimd.memset(ones_sb[:], 1.0)
            neg1_sb = consts.tile([128, 1], F32)
            nc.gpsimd.memset(neg1_sb[:], -1.0)
            if affine:
                lnw_sb = consts.tile([128, NIC], F32)
                nc.sync.dma_start(lnw_sb[:], lnw_ext.rearrange("(f p) -> p f", p=128))
                lnb_sb = consts.tile([128, NIC], F32)
                nc.sync.dma_start(lnb_sb[:], lnb_ext.rearrange("(f p) -> p f", p=128))

            ptiles = {}     # (pair, m) -> [128,1024] bf16 pair-tile
            bc_map = {}     # chunk -> broadcast tile
            wgq = {}        # (ot, g) -> prefetched weight tile

            def rhs(s):
                pair, r = divmod(s, 2 * NM)
                m, sub = divmod(r, 2)
                return ptiles[(pair, m)][:, ds(sub * TCH, TCH)]

            def wg_get(ot, g):
                key = (ot, g)
                if key in wgq:
                    return wgq.pop(key)
                w_ = wp.tile([128, KG, 128], BF16, tag="wg")
                nc.sync.dma_start(w_[:], wt_ext[ot, g])
                return w_

            def wg_prefetch(ot, g):
                w_ = wp.tile([128, KG, 128], BF16, tag="wg")
                nc.sync.dma_start(w_[:], wt_ext[ot, g])
                wgq[(ot, g)] = w_

            def stats_steps(c):
                """Closures: 16 per-ic steps + 1 rows/broadcast step."""
                stp = statps.tile([128, TCH], F32, tag="st", name=f"st{c}")

                def ic_step(ic):
                    def f():
                        xs = xsp.tile([128, TCH], F32, tag="xs")
                        nc.sync.dma_start(
                            xs[:], xt_ext[ds(ic * 128, 128), ds(c * TCH, TCH)]
                        )
                        sq = sqp.tile([128, TCH], F32, tag="sq")
                        nc.scalar.square(sq[:], xs[:])
                        nc.tensor.matmul(stp[0:1, :], ones_sb[:], xs[:],
                                         start=(ic == 0), stop=(ic == NIC - 1))
                        nc.tensor.matmul(stp[32:33, :], ones_sb[:], sq[:],
                                         start=(ic == 0), stop=(ic == NIC - 1))
                    return f

                def rows_step():
                    mean = rowp.tile([1, TCH], F32, tag="mean", name=f"mean{c}")
                    nc.vector.tensor_scalar_mul(mean[:], stp[0:1, :], 1.0 / I)
                    var = rowp.tile([1, TCH], F32, tag="var", name=f"var{c}")
                    nc.vector.tensor_scalar_mul(var[:], stp[32:33, :], 1.0 / I)
                    row = rowp.tile([1, 2 * TCH], F32, tag="row", name=f"row{c}")
                    rr = row[0:1, 0:TCH]
                    nc.vector.tensor_tensor(rr, mean[:], mean[:], ALU.mult)
                    nc.vector.tensor_sub(var[:], var[:], rr)
                    # std in place of var, istd into the row's first half
                    nc.scalar.activation(var[:], var[:], AF.Sqrt, bias=eps_sb[:])
                    nc.vector.reciprocal(rr, var[:])
                    nc.vector.scalar_tensor_tensor(
                        row[0:1, TCH:2 * TCH], mean[:], -1.0, rr,
                        ALU.mult, ALU.mult,
                    )
                    bc = bcp.tile([128, 2 * TCH], F32, tag="bc", name=f"bc{c}")
                    nc.gpsimd.partition_broadcast(bc[:], row[0:1, :])
                    bc_map[c] = bc

                return [ic_step(ic) for ic in range(NIC)] + [rows_step]

            def gen_chunk(c, deferred=None):
                """Panel generation for chunk c; race matmuls interleaved.
                `deferred` emits the previous chunk's last o-tile drain
                after pair 0 (keeps DVE free to pre-generate pair 0)."""
                bc = bc_map.pop(c)
                iv = bc[:, 0:TCH].unsqueeze(1).broadcast_to((128, 2, TCH))
                nv = bc[:, TCH:2 * TCH].unsqueeze(1).broadcast_to((128, 2, TCH))
                pss = [
                    mmps.tile([128, TCH], F32, tag="ps", name=f"rps{r}_{c}")
                    for r in range(N_RACE)
                ]
                g_next = 0

                def race_mm(g_hi):
                    nonlocal g_next
                    for g in range(g_next, g_hi):
                        for r in range(N_RACE):
                            w_ = wg_get(r, g)
                            for ks in range(KG):
                                s = g * KG + ks
                                nc.tensor.matmul(
                                    pss[r][:], w_[:, ks, :], rhs(s),
                                    start=(s == 0), stop=(s == NK - 1),
                                )
                    g_next = g_hi

                for pair in range(NP):
                    xg = xgp.tile([128, 2 * TCH], F32, tag="xg",
                                  name=f"xg_{c}_{pair}")
                    for sub in range(2):
                        ic = 2 * pair + sub
                        nc.sync.dma_start(
                            xg[:, ds(sub * TCH, TCH)],
                            xt_ext[ds(ic * 128, 128), ds(c * TCH, TCH)],
                        )
                    # normalize in place: xn = x*istd + (-mu*istd), per-token
                    nc.vector.tensor_tensor(xg[:], xg[:], iv, ALU.mult)
                    nc.vector.tensor_tensor(xg[:], xg[:], nv, ALU.add)
                    if affine:
                        for sub in range(2):
                            ic = 2 * pair + sub
                            nc.scalar.activation(
                                xg[:, ds(sub * TCH, TCH)], xg[:, ds(sub * TCH, TCH)],
                                AF.Identity,
                                bias=lnb_sb[:, ic:ic + 1], scale=lnw_sb[:, ic:ic + 1],
                            )

                    def pt(m):
                        t_ = panelp.tile([128, 2 * TCH], BF16, tag=f"p{pair}_{m}",
                                         name=f"pan_{c}_{pair}_{m}")
                        ptiles[(pair, m)] = t_
                        return t_

                    th = xgp.tile([128, 2 * TCH], F32, tag="xg",
                                  name=f"th_{c}_{pair}")
                    nc.scalar.activation(th[:], xg[:], AF.Tanh)
                    p0 = pt(0)
                    nc.scalar.activation(p0[:], xg[:], AF.Silu)
                    sh = shp.tile([128, 2 * TCH], F32, tag="sh")
                    nc.scalar.activation(sh[:], th[:], AF.Sin, scale=math.pi / 2)

                    p1, p2, p3, p4 = pt(1), pt(2), pt(3), pt(4)
                    p5, p6, p7, p8 = pt(5), pt(6), pt(7), pt(8)
                    for sub in range(2):
                        hs = ds(sub * TCH, TCH)
                        shh = sh[:, hs]

                        def scr(tag_i):
                            return scrp.tile([128, TCH], F32, tag="scr",
                                             name=f"scr{tag_i}_{c}_{pair}_{sub}")

                        def lad(tag):
                            return ladp.tile([128, TCH], F32, tag=tag,
                                             name=f"lad_{tag}_{c}_{pair}_{sub}")

                        # u = -2*sh^2 ; c1 = u + 1  (c1 on ACT)
                        u = scr("u")
                        nc.vector.scalar_tensor_tensor(u[:], shh, -2.0, shh,
                                                       ALU.mult, ALU.mult)
                        c1 = lad("c1")
                        nc.scalar.activation(c1[:], u[:], AF.Identity, bias=1.0)
                        sq1 = scr("s1")
                        nc.scalar.square(sq1[:], c1[:])
                        c2 = lad("c2")
                        nc.vector.tensor_scalar(c2[:], sq1[:], 2.0, -1.0,
                                                ALU.mult, ALU.add)
                        u3 = scr("u3")
                        nc.vector.scalar_tensor_tensor(u3[:], c2[:], 2.0, c1[:],
                                                       ALU.mult, ALU.mult)
                        c3 = lad("c3")
                        nc.vector.tensor_sub(c3[:], u3[:], c1[:])
                        sq2 = scr("s2")
                        nc.scalar.square(sq2[:], c2[:])
                        c4 = lad("c4")
                        nc.vector.tensor_scalar(c4[:], sq2[:], 2.0, -1.0,
                                                ALU.mult, ALU.add)
                        # exports m=1..4
                        nc.scalar.copy(p1[:, hs], c1[:])
                        nc.scalar.copy(p2[:, hs], c2[:])
                        nc.scalar.copy(p3[:, hs], c3[:])
                        nc.vector.tensor_copy(p4[:, hs], c4[:])
                        # leaves m=5..8 straight to panel halves
                        u5 = scr("u5")
                        nc.vector.scalar_tensor_tensor(u5[:], c3[:], 2.0, c2[:],
                                                       ALU.mult, ALU.mult)
                        nc.vector.tensor_sub(p5[:, hs], u5[:], c1[:])
                        sq3 = scr("s3")
                        nc.scalar.square(sq3[:], c3[:])
                        nc.scalar.activation(p6[:, hs], sq3[:], AF.Identity,
                                             bias=neg1_sb[:], scale=2.0)
                        u7 = scr("u7")
                        nc.vector.scalar_tensor_tensor(u7[:], c4[:], 2.0, c3[:],
                                                       ALU.mult, ALU.mult)
                        nc.vector.tensor_sub(p7[:, hs], u7[:], c1[:])
                        sq4 = scr("s4")
                        nc.scalar.square(sq4[:], c4[:])
                        nc.scalar.activation(p8[:, hs], sq4[:], AF.Identity,
                                             bias=neg1_sb[:], scale=2.0)

                    if pair == 0 and deferred is not None:
                        deferred()
                    race_mm((2 * NM * (pair + 1)) // KG)
                race_mm(NG)
                for r in range(N_RACE):
                    stg = stgp.tile([128, TCH], BF16, tag="stg",
                                    name=f"rstg{r}_{c}")
                    nc.vector.tensor_copy(stg[:], pss[r][:])
                    nc.sync.dma_start(
                        out_ext[ds(r * 128, 128), ds(c * TCH, TCH)], stg[:]
                    )

            def mm_chunk(c, steps):
                """O-tiles N_RACE..15; next-chunk stats steps injected
                spread over o-tile indices 2..5.  Returns a deferred
                closure for the last o-tile's drain (or None)."""
                n_ots = NOT - N_RACE
                deferred = None
                for oi, ot in enumerate(range(N_RACE, NOT)):
                    ps = mmps.tile([128, TCH], F32, tag="ps", name=f"mps{c}_{ot}")
                    for g in range(NG):
                        if steps and oi >= 2 and (g % 4 == 1):
                            steps.pop(0)()
                        w_ = wg_get(ot, g)
                        for ks in range(KG):
                            s = g * KG + ks
                            nc.tensor.matmul(
                                ps[:], w_[:, ks, :], rhs(s),
                                start=(s == 0), stop=(s == NK - 1),
                            )

                    def drain(ps=ps, ot=ot):
                        stg = stgp.tile([128, TCH], BF16, tag="stg",
                                        name=f"stg{c}_{ot}")
                        nc.vector.tensor_copy(stg[:], ps[:])
                        nc.sync.dma_start(
                            out_ext[ds(ot * 128, 128), ds(c * TCH, TCH)], stg[:]
                        )

                    if oi == n_ots - 1 and c + 1 < NCH:
                        deferred = drain
                    else:
                        drain()
                while steps:
                    steps.pop(0)()
                return deferred

            # --- program ---
            for g in range(2):
                for r in range(N_RACE):
                    if len(wgq) < 4:
                        wg_prefetch(r, g)
            for f in stats_steps(0):
                f()
            deferred = None
            for c in range(NCH):
                gen_chunk(c, deferred=deferred)
                steps = stats_steps(c + 1) if c + 1 < NCH else []
                deferred = mm_chunk(c, steps)

    nc.compile()
    return nc


def prep_weights(base_weight, spline_weight):
    """Host-side: bf16 W_all in pair-interleaved k-step order
    (s = pair*18 + 2m + sub), pre-tiled for [128, KG, 128] DMAs:
    wt[ot, g, k_in, ks, o_in]."""
    w = np.empty((NM, I, O), np.float32)
    w[0] = base_weight.T                      # [i, o]
    for g in range(G):
        w[1 + g] = spline_weight[:, :, g].T   # [i, o]
    w = w.reshape(NM, NP, 2, 128, O).transpose(1, 0, 2, 3, 4)  # [pair, m, sub, 128, o]
    w = w.reshape(NK * 128, O)
    w = w.reshape(NG, KG, 128, NOT, 128).transpose(3, 0, 2, 1, 4)
    return np.ascontiguousarray(w.astype(ml_dtypes.bfloat16))


_NC_CACHE = {}


def _get_nc(affine=False):
    if affine not in _NC_CACHE:
        _NC_CACHE[affine] = build_nc(affine=affine)
    return _NC_CACHE[affine]


def kernel(x, ln_weight, ln_bias, base_weight, spline_weight):
    x = np.asarray(x, np.float32)
    ln_weight = np.asarray(ln_weight, np.float32)
    ln_bias = np.asarray(ln_bias, np.float32)
    affine = not (np.all(ln_weight == 1.0) and np.all(ln_bias == 0.0))
    wt = prep_weights(np.asarray(base_weight, np.float32),
                      np.asarray(spline_weight, np.float32))
    nc = _get_nc(affine)
    in_maps = [
        {
            "xt": np.ascontiguousarray(x[b].T),
            "lnw": ln_weight,
            "lnb": ln_bias,
            "wt": wt,
        }
        for b in range(B)
    ]
    res = run_bass_kernel_spmd(nc, in_maps, core_ids=list(range(B)))
    out = np.stack([res.results[b]["out"].astype(np.float32).T for b in range(B)])
    return np.ascontiguousarray(out)


# revision 15
# speedup vs baseline: 1.0360x; 1.0136x over previous
"""Trainium2 Bass kernel for AdvancedKANLayer (v2).

Math (per reference):
  xn    = LayerNorm(x) * ln_w + ln_b           (eps=1e-5)
  base  = silu(xn) @ base_weight.T             [B,S,O]
  t     = tanh(xn)
  basis = cos(pi*k*t), k=1..8
  spl   = einsum('bsig,oig->bso', basis, spline_weight)
  out   = base + spl

Strategy: data-parallel over batch (8 cores, one batch entry each, no
collectives).  Per core the whole thing is one K=18432 GEMM:
  out[o, t] = sum_k W_all[k, o] * panel[k, t]
where panel rows are [silu(xn); cos(1*pi*t); ...; cos(8*pi*t)] per
I-chunk, generated on-chip via a Chebyshev ladder from
c1 = cos(pi*t) = 1 - 2*sin(pi*t/2)^2 (ScalarE Sin valid on [-pi,pi]).

v2 changes vs v1:
 - x arrives HOST-TRANSPOSED as xt [I, T]: panel generation reads
   i-major tiles straight from DRAM; the 256 PE transposes are gone.
 - LN stats per token via fp32 ones-matmuls on TensorE (column sums of
   x and x^2 accumulated over the 16 i-blocks into one PSUM bank at
   partitions 0/32), tiny row math on DVE, then one gpsimd
   partition_broadcast of [istd | -mu*istd] rows.  Normalization is two
   DVE tensor_tensor ops with free-dim-broadcast APs.
 - k-step order interleaves i-block PAIRS (s = pair*18 + 2m + sub) so
   tanh/silu/sin run at [128,1024] (half the ACT dispatch overhead);
   panel tiles are [128,1024] pair-tiles, matmuls consume 512-halves.
 - KG=8 weight groups (2KB DMA lines, half the issue count), wt bf16.
 - Output written bf16 (host upcasts): half the out-DMA bytes.
 - n_race=5 o-tiles race panel generation; ladder ops split between
   ACT and DVE to balance both near ~120us/chunk.
"""

import math

import numpy as np
import ml_dtypes

import concourse.bass as bass
import concourse.mybir as mybir
import concourse.tile as tile
from concourse import bacc
from concourse.bass import ds, ts
from concourse.bass_utils import run_bass_kernel_spmd

F32 = mybir.dt.float32
BF16 = mybir.dt.bfloat16
AF = mybir.ActivationFunctionType
ALU = mybir.AluOpType

EPS = 1e-5

# geometry (full problem, per core)
B = 8
T = 2048          # tokens per core (= S, one batch entry per core)
I = 2048          # input dim
O = 2048          # output dim
G = 8             # cos harmonics
TCH = 512         # token chunk (matmul N)
NCH = T // TCH    # 4
NIC = I // 128    # 16 I-chunks
NP = NIC // 2     # 8 pairs
NM = G + 1        # 9 panel row-groups per ic (silu + 8 cos)
NK = NIC * NM     # 144 k-steps of 128
KG = 8            # k-steps per weight DMA group
NG = NK // KG     # 18
NOT = O // 128    # 16 o-tiles
N_RACE = 5        # o-tiles racing panel generation


def build_nc(affine=False):
    nc = bacc.Bacc("TRN2", target_bir_lowering=False, debug=False)
    xt_ext = nc.declare_dram_parameter("xt", [I, T], F32, isOutput=False)
    lnw_ext = nc.declare_dram_parameter("lnw", [I], F32, isOutput=False)
    lnb_ext = nc.declare_dram_parameter("lnb", [I], F32, isOutput=False)
    wt_ext = nc.declare_dram_parameter("wt", [NOT, NG, 128, KG, 128], BF16, isOutput=False)
    out_ext = nc.declare_dram_parameter("out", [O, T], BF16, isOutput=True)

    with tile.TileContext(nc) as tc:
        with (
            tc.tile_pool(name="consts", bufs=1) as consts,
            tc.tile_pool(name="xsp", bufs=2) as xsp,       # stats x stream [128,512] f32
            tc.tile_pool(name="sqp", bufs=2) as sqp,       # squares [128,512] f32
            tc.tile_pool(name="xgp", bufs=3) as xgp,       # gen pair tiles + tanh scratch [128,1024] f32
            tc.tile_pool(name="shp", bufs=1) as shp,       # sin tile [128,1024] f32
            tc.tile_pool(name="scrp", bufs=3) as scrp,     # ladder scratch [128,512] f32
            tc.tile_pool(name="ladp", bufs=1) as ladp,     # c1..c4 [128,512] f32
            tc.tile_pool(name="rowp", bufs=1) as rowp,     # stat rows [1,*] f32
            tc.tile_pool(name="bcp", bufs=1) as bcp,       # broadcast [128,1024] f32
            tc.tile_pool(name="panelp", bufs=1) as panelp, # 72 pair-tiles [128,1024] bf16
            tc.tile_pool(name="wp", bufs=5) as wp,         # weights [128,KG,128] bf16
            tc.tile_pool(name="stgp", bufs=2) as stgp,     # out staging [128,512] bf16
            tc.tile_pool(name="statps", bufs=1, space="PSUM") as statps,
            tc.tile_pool(name="mmps", bufs=7, space="PSUM") as mmps,
        ):
            eps_sb = consts.tile([1, 1], F32)
            nc.gpsimd.memset(eps_sb[:], EPS)
            ones_sb = consts.tile([128, 1], F32)
            nc.gpsimd.memset(ones_sb[:], 1.0)
            neg1_sb = consts.tile([128, 1], F32)
            nc.gpsimd.memset(neg1_sb[:], -1.0)
            ones_bf = consts.tile([128, 1], BF16)
            nc.gpsimd.memset(ones_bf[:], 1.0)
            if affine:
                lnw_sb = consts.tile([128, NIC], F32)
                nc.sync.dma_start(lnw_sb[:], lnw_ext.rearrange("(f p) -> p f", p=128))
                lnb_sb = consts.tile([128, NIC], F32)
                nc.sync.dma_start(lnb_sb[:], lnb_ext.rearrange("(f p) -> p f", p=128))

            ptiles = {}     # (pair, m) -> [128,1024] bf16 pair-tile
            bc_map = {}     # chunk -> broadcast tile
            wgq = {}        # (ot, g) -> prefetched weight tile

            def rhs(s):
                pair, r = divmod(s, 2 * NM)
                m, sub = divmod(r, 2)
                return ptiles[(pair, m)][:, ds(sub * TCH, TCH)]

            def wg_get(ot, g):
                key = (ot, g)
                if key in wgq:
                    return wgq.pop(key)
                w_ = wp.tile([128, KG, 128], BF16, tag="wg")
                nc.sync.dma_start(w_[:], wt_ext[ot, g])
                return w_

            def wg_prefetch(ot, g):
                w_ = wp.tile([128, KG, 128], BF16, tag="wg")
                nc.sync.dma_start(w_[:], wt_ext[ot, g])
                wgq[(ot, g)] = w_

            def stats_steps(c):
                """Closures: 16 per-ic steps + 1 rows/broadcast step."""
                stp = statps.tile([128, TCH], F32, tag="st", name=f"st{c}")

                def ic_step(ic):
                    def f():
                        xs = xsp.tile([128, TCH], F32, tag="xs")
                        nc.gpsimd.dma_start(
                            xs[:], xt_ext[ds(ic * 128, 128), ds(c * TCH, TCH)]
                        )
                        sq = sqp.tile([128, TCH], BF16, tag="sq")
                        nc.scalar.square(sq[:], xs[:])
                        nc.tensor.matmul(stp[0:1, :], ones_sb[:], xs[:],
                                         start=(ic == 0), stop=(ic == NIC - 1))
                        nc.tensor.matmul(stp[32:33, :], ones_bf[:], sq[:],
                                         start=(ic == 0), stop=(ic == NIC - 1))
                    return f

                def rows_step():
                    mean = rowp.tile([1, TCH], F32, tag="mean", name=f"mean{c}")
                    nc.vector.tensor_scalar_mul(mean[:], stp[0:1, :], 1.0 / I)
                    var = rowp.tile([1, TCH], F32, tag="var", name=f"var{c}")
                    nc.vector.tensor_scalar_mul(var[:], stp[32:33, :], 1.0 / I)
                    row = rowp.tile([1, 2 * TCH], F32, tag="row", name=f"row{c}")
                    rr = row[0:1, 0:TCH]
                    nc.vector.tensor_tensor(rr, mean[:], mean[:], ALU.mult)
                    nc.vector.tensor_sub(var[:], var[:], rr)
                    # std in place of var, istd into the row's first half
                    nc.scalar.activation(var[:], var[:], AF.Sqrt, bias=eps_sb[:])
                    nc.vector.reciprocal(rr, var[:])
                    nc.vector.scalar_tensor_tensor(
                        row[0:1, TCH:2 * TCH], mean[:], -1.0, rr,
                        ALU.mult, ALU.mult,
                    )
                    bc = bcp.tile([128, 2 * TCH], F32, tag="bc", name=f"bc{c}")
                    nc.gpsimd.partition_broadcast(bc[:], row[0:1, :])
                    bc_map[c] = bc

                return [ic_step(ic) for ic in range(NIC)] + [rows_step]

            def gen_chunk(c, deferred=None):
                """Panel generation for chunk c; race matmuls interleaved.
                `deferred` emits the previous chunk's last o-tile drain
                after pair 0 (keeps DVE free to pre-generate pair 0)."""
                bc = bc_map.pop(c)
                iv = bc[:, 0:TCH].unsqueeze(1).broadcast_to((128, 2, TCH))
                nv = bc[:, TCH:2 * TCH].unsqueeze(1).broadcast_to((128, 2, TCH))
                pss = [
                    mmps.tile([128, TCH], F32, tag="ps", name=f"rps{r}_{c}")
                    for r in range(N_RACE)
                ]
                g_next = 0

                def race_mm(g_hi):
                    nonlocal g_next
                    for g in range(g_next, g_hi):
                        for r in range(N_RACE):
                            w_ = wg_get(r, g)
                            for ks in range(KG):
                                s = g * KG + ks
                                nc.tensor.matmul(
                                    pss[r][:], w_[:, ks, :], rhs(s),
                                    start=(s == 0), stop=(s == NK - 1),
                                )
                    g_next = g_hi

                for pair in range(NP):
                    xg = xgp.tile([128, 2 * TCH], F32, tag="xg",
                                  name=f"xg_{c}_{pair}")
                    for sub in range(2):
                        ic = 2 * pair + sub
                        nc.gpsimd.dma_start(
                            xg[:, ds(sub * TCH, TCH)],
                            xt_ext[ds(ic * 128, 128), ds(c * TCH, TCH)],
                        )
                    # normalize in place: xn = x*istd + (-mu*istd), per-token
                    nc.vector.tensor_tensor(xg[:], xg[:], iv, ALU.mult)
                    nc.vector.tensor_tensor(xg[:], xg[:], nv, ALU.add)
                    if affine:
                        for sub in range(2):
                            ic = 2 * pair + sub
                            nc.scalar.activation(
                                xg[:, ds(sub * TCH, TCH)], xg[:, ds(sub * TCH, TCH)],
                                AF.Identity,
                                bias=lnb_sb[:, ic:ic + 1], scale=lnw_sb[:, ic:ic + 1],
                            )

                    def pt(m):
                        t_ = panelp.tile([128, 2 * TCH], BF16, tag=f"p{pair}_{m}",
                                         name=f"pan_{c}_{pair}_{m}")
                        ptiles[(pair, m)] = t_
                        return t_

                    th = xgp.tile([128, 2 * TCH], F32, tag="xg",
                                  name=f"th_{c}_{pair}")
                    nc.scalar.activation(th[:], xg[:], AF.Tanh)
                    p0 = pt(0)
                    nc.scalar.activation(p0[:], xg[:], AF.Silu)
                    sh = shp.tile([128, 2 * TCH], F32, tag="sh")
                    nc.scalar.activation(sh[:], th[:], AF.Sin, scale=math.pi / 2)

                    p1, p2, p3, p4 = pt(1), pt(2), pt(3), pt(4)
                    p5, p6, p7, p8 = pt(5), pt(6), pt(7), pt(8)
                    for sub in range(2):
                        hs = ds(sub * TCH, TCH)
                        shh = sh[:, hs]

                        def scr(tag_i):
                            return scrp.tile([128, TCH], F32, tag="scr",
                                             name=f"scr{tag_i}_{c}_{pair}_{sub}")

                        def lad(tag):
                            return ladp.tile([128, TCH], F32, tag=tag,
                                             name=f"lad_{tag}_{c}_{pair}_{sub}")

                        # u = -2*sh^2 ; c1 = u + 1  (c1 on ACT)
                        u = scr("u")
                        nc.vector.scalar_tensor_tensor(u[:], shh, -2.0, shh,
                                                       ALU.mult, ALU.mult)
                        c1 = lad("c1")
                        nc.vector.tensor_scalar_add(c1[:], u[:], 1.0)
                        sq1 = scr("s1")
                        nc.scalar.square(sq1[:], c1[:])
                        c2 = lad("c2")
                        nc.vector.tensor_scalar(c2[:], sq1[:], 2.0, -1.0,
                                                ALU.mult, ALU.add)
                        u3 = scr("u3")
                        nc.vector.scalar_tensor_tensor(u3[:], c2[:], 2.0, c1[:],
                                                       ALU.mult, ALU.mult)
                        c3 = lad("c3")
                        nc.vector.tensor_sub(c3[:], u3[:], c1[:])
                        sq2 = scr("s2")
                        nc.scalar.square(sq2[:], c2[:])
                        c4 = lad("c4")
                        nc.vector.tensor_scalar(c4[:], sq2[:], 2.0, -1.0,
                                                ALU.mult, ALU.add)
                        # exports m=1..4
                        nc.scalar.copy(p1[:, hs], c1[:])
                        nc.scalar.copy(p2[:, hs], c2[:])
                        nc.scalar.copy(p3[:, hs], c3[:])
                        nc.vector.tensor_copy(p4[:, hs], c4[:])
                        # leaves m=5..8 straight to panel halves
                        u5 = scr("u5")
                        nc.vector.scalar_tensor_tensor(u5[:], c3[:], 2.0, c2[:],
                                                       ALU.mult, ALU.mult)
                        nc.vector.tensor_sub(p5[:, hs], u5[:], c1[:])
                        sq3 = scr("s3")
                        nc.scalar.square(sq3[:], c3[:])
                        nc.scalar.activation(p6[:, hs], sq3[:], AF.Identity,
                                             bias=neg1_sb[:], scale=2.0)
                        u7 = scr("u7")
                        nc.vector.scalar_tensor_tensor(u7[:], c4[:], 2.0, c3[:],
                                                       ALU.mult, ALU.mult)
                        nc.vector.tensor_sub(p7[:, hs], u7[:], c1[:])
                        sq4 = scr("s4")
                        nc.scalar.square(sq4[:], c4[:])
                        nc.scalar.activation(p8[:, hs], sq4[:], AF.Identity,
                                             bias=neg1_sb[:], scale=2.0)

                    if pair == 0 and deferred is not None:
                        deferred()
                    race_mm((2 * NM * (pair + 1)) // KG)
                race_mm(NG)
                for r in range(N_RACE):
                    stg = stgp.tile([128, TCH], BF16, tag="stg",
                                    name=f"rstg{r}_{c}")
                    nc.vector.tensor_copy(stg[:], pss[r][:])
                    nc.gpsimd.dma_start(
                        out_ext[ds(r * 128, 128), ds(c * TCH, TCH)], stg[:]
                    )

            def mm_chunk(c, steps):
                """O-tiles N_RACE..15; next-chunk stats steps injected
                spread over o-tile indices 2..5.  Returns a deferred
                closure for the last o-tile's drain (or None)."""
                n_ots = NOT - N_RACE
                deferred = None
                for oi, ot in enumerate(range(N_RACE, NOT)):
                    ps = mmps.tile([128, TCH], F32, tag="ps", name=f"mps{c}_{ot}")
                    for g in range(NG):
                        if steps and (g % 3 == 1):
                            steps.pop(0)()
                        w_ = wg_get(ot, g)
                        for ks in range(KG):
                            s = g * KG + ks
                            nc.tensor.matmul(
                                ps[:], w_[:, ks, :], rhs(s),
                                start=(s == 0), stop=(s == NK - 1),
                            )

                    def drain(ps=ps, ot=ot):
                        stg = stgp.tile([128, TCH], BF16, tag="stg",
                                        name=f"stg{c}_{ot}")
                        nc.vector.tensor_copy(stg[:], ps[:])
                        nc.gpsimd.dma_start(
                            out_ext[ds(ot * 128, 128), ds(c * TCH, TCH)], stg[:]
                        )

                    if oi == n_ots - 1 and c + 1 < NCH:
                        deferred = drain
                    else:
                        drain()
                while steps:
                    steps.pop(0)()
                return deferred

            # --- program ---
            for g in range(2):
                for r in range(N_RACE):
                    if len(wgq) < 5:
                        wg_prefetch(r, g)
            for f in stats_steps(0):
                f()
            deferred = None
            for c in range(NCH):
                gen_chunk(c, deferred=deferred)
                steps = stats_steps(c + 1) if c + 1 < NCH else []
                deferred = mm_chunk(c, steps)

    nc.compile()
    return nc


def prep_weights(base_weight, spline_weight):
    """Host-side: bf16 W_all in pair-interleaved k-step order
    (s = pair*18 + 2m + sub), pre-tiled for [128, KG, 128] DMAs:
    wt[ot, g, k_in, ks, o_in]."""
    w = np.empty((NM, I, O), np.float32)
    w[0] = base_weight.T                      # [i, o]
    for g in range(G):
        w[1 + g] = spline_weight[:, :, g].T   # [i, o]
    w = w.reshape(NM, NP, 2, 128, O).transpose(1, 0, 2, 3, 4)  # [pair, m, sub, 128, o]
    w = w.reshape(NK * 128, O)
    w = w.reshape(NG, KG, 128, NOT, 128).transpose(3, 0, 2, 1, 4)
    return np.ascontiguousarray(w.astype(ml_dtypes.bfloat16))


_NC_CACHE = {}


def _get_nc(affine=False):
    if affine not in _NC_CACHE:
        _NC_CACHE[affine] = build_nc(affine=affine)
    return _NC_CACHE[affine]


def kernel(x, ln_weight, ln_bias, base_weight, spline_weight):
    x = np.asarray(x, np.float32)
    ln_weight = np.asarray(ln_weight, np.float32)
    ln_bias = np.asarray(ln_bias, np.float32)
    affine = not (np.all(ln_weight == 1.0) and np.all(ln_bias == 0.0))
    wt = prep_weights(np.asarray(base_weight, np.float32),
                      np.asarray(spline_weight, np.float32))
    nc = _get_nc(affine)
    in_maps = [
        {
            "xt": np.ascontiguousarray(x[b].T),
            "lnw": ln_weight,
            "lnb": ln_bias,
            "wt": wt,
        }
        for b in range(B)
    ]
    res = run_bass_kernel_spmd(nc, in_maps, core_ids=list(range(B)))
    out = np.stack([res.results[b]["out"].astype(np.float32).T for b in range(B)])
    return np.ascontiguousarray(out)


# revision 27
# speedup vs baseline: 1.0813x; 1.0437x over previous
"""Trainium2 Bass kernel for AdvancedKANLayer (v2).

Math (per reference):
  xn    = LayerNorm(x) * ln_w + ln_b           (eps=1e-5)
  base  = silu(xn) @ base_weight.T             [B,S,O]
  t     = tanh(xn)
  basis = cos(pi*k*t), k=1..8
  spl   = einsum('bsig,oig->bso', basis, spline_weight)
  out   = base + spl

Strategy: data-parallel over batch (8 cores, one batch entry each, no
collectives).  Per core the whole thing is one K=18432 GEMM:
  out[o, t] = sum_k W_all[k, o] * panel[k, t]
where panel rows are [silu(xn); cos(1*pi*t); ...; cos(8*pi*t)] per
I-chunk, generated on-chip via a Chebyshev ladder from
c1 = cos(pi*t) = 1 - 2*sin(pi*t/2)^2 (ScalarE Sin valid on [-pi,pi]).

v2 changes vs v1:
 - x arrives HOST-TRANSPOSED as xt [I, T]: panel generation reads
   i-major tiles straight from DRAM; the 256 PE transposes are gone.
 - LN stats per token via fp32 ones-matmuls on TensorE (column sums of
   x and x^2 accumulated over the 16 i-blocks into one PSUM bank at
   partitions 0/32), tiny row math on DVE, then one gpsimd
   partition_broadcast of [istd | -mu*istd] rows.  Normalization is two
   DVE tensor_tensor ops with free-dim-broadcast APs.
 - k-step order interleaves i-block PAIRS (s = pair*18 + 2m + sub) so
   tanh/silu/sin run at [128,1024] (half the ACT dispatch overhead);
   panel tiles are [128,1024] pair-tiles, matmuls consume 512-halves.
 - KG=8 weight groups (2KB DMA lines, half the issue count), wt bf16.
 - Output written bf16 (host upcasts): half the out-DMA bytes.
 - n_race=5 o-tiles race panel generation; ladder ops split between
   ACT and DVE to balance both near ~120us/chunk.
"""

import math

import numpy as np
import ml_dtypes

import concourse.bass as bass
import concourse.mybir as mybir
import concourse.tile as tile
from concourse import bacc
from concourse.bass import ds, ts
from concourse.bass_utils import run_bass_kernel_spmd

F32 = mybir.dt.float32
BF16 = mybir.dt.bfloat16
FP8E5 = mybir.dt.float8e5
PM = mybir.MatmulPerfMode
AF = mybir.ActivationFunctionType
ALU = mybir.AluOpType

EPS = 1e-5

# geometry (full problem, per core)
B = 8
T = 2048          # tokens per core (= S, one batch entry per core)
I = 2048          # input dim
O = 2048          # output dim
G = 8             # cos harmonics
TCH = 512         # token chunk (matmul N)
NCH = T // TCH    # 4
NIC = I // 128    # 16 I-chunks
NP = NIC // 2     # 8 pairs
NM = G + 1        # 9 panel row-groups per ic (silu + 8 cos)
NKB = NIC * G     # 128 bf16 (cos) k-steps of 128; silu goes fp8 DoubleRow
KG = 8            # k-steps per weight DMA group
NG = NKB // KG    # 16
NOT = O // 128    # 16 o-tiles
N_RACE = 5        # o-tiles racing panel generation


def build_nc(affine=False):
    nc = bacc.Bacc("TRN2", target_bir_lowering=False, debug=False)
    xt_ext = nc.declare_dram_parameter("xt", [I, T], F32, isOutput=False)
    lnw_ext = nc.declare_dram_parameter("lnw", [I], F32, isOutput=False)
    lnb_ext = nc.declare_dram_parameter("lnb", [I], F32, isOutput=False)
    wt_ext = nc.declare_dram_parameter("wt", [NOT, NG, 128, KG, 128], BF16, isOutput=False)
    wt0_ext = nc.declare_dram_parameter("wt0", [NOT, 128, NP, 2, 128], FP8E5, isOutput=False)
    out_ext = nc.declare_dram_parameter("out", [O, T], BF16, isOutput=True)

    with tile.TileContext(nc) as tc:
        with (
            tc.tile_pool(name="consts", bufs=1) as consts,
            tc.tile_pool(name="xsp", bufs=2) as xsp,       # stats x stream [128,512] f32
            tc.tile_pool(name="sqp", bufs=2) as sqp,       # squares [128,512] f32
            tc.tile_pool(name="xgp", bufs=3) as xgp,       # gen pair tiles + tanh scratch [128,1024] f32
            tc.tile_pool(name="shp", bufs=1) as shp,       # sin tile [128,1024] f32
            tc.tile_pool(name="scrp", bufs=3) as scrp,     # ladder scratch [128,512] f32
            tc.tile_pool(name="ladp", bufs=1) as ladp,     # c1..c4 [128,512] f32
            tc.tile_pool(name="rowp", bufs=1) as rowp,     # stat rows [1,*] f32
            tc.tile_pool(name="bcp", bufs=1) as bcp,       # broadcast [128,1024] f32
            tc.tile_pool(name="panelp", bufs=1) as panelp, # 72 pair-tiles [128,1024] bf16
            tc.tile_pool(name="wp", bufs=5) as wp,         # weights [128,KG,128] bf16
            tc.tile_pool(name="wdrp", bufs=2) as wdrp,     # DR silu weights [128,NP,2,128] fp8
            tc.tile_pool(name="stgp", bufs=2) as stgp,     # out staging [128,512] bf16
            tc.tile_pool(name="statps", bufs=1, space="PSUM") as statps,
            tc.tile_pool(name="mmps", bufs=7, space="PSUM") as mmps,
        ):
            eps_sb = consts.tile([1, 1], F32)
            nc.gpsimd.memset(eps_sb[:], EPS)
            ones_sb = consts.tile([128, 1], F32)
            nc.gpsimd.memset(ones_sb[:], 1.0)
            neg1_sb = consts.tile([128, 1], F32)
            nc.gpsimd.memset(neg1_sb[:], -1.0)
            ones_bf = consts.tile([128, 1], BF16)
            nc.gpsimd.memset(ones_bf[:], 1.0)
            if affine:
                lnw_sb = consts.tile([128, NIC], F32)
                nc.sync.dma_start(lnw_sb[:], lnw_ext.rearrange("(f p) -> p f", p=128))
                lnb_sb = consts.tile([128, NIC], F32)
                nc.sync.dma_start(lnb_sb[:], lnb_ext.rearrange("(f p) -> p f", p=128))

            ptiles = {}     # (pair, m) -> [128,1024] bf16 pair-tile
            bc_map = {}     # chunk -> broadcast tile
            wgq = {}        # (ot, g) -> prefetched weight tile

            def rhs(sp, q=None):
                """bf16 cos k-step sp in [0,128): pair-major, m=1..8."""
                pair, r = divmod(sp, 2 * G)
                m, sub = divmod(r, 2)
                m += 1
                if q is None:
                    return ptiles[(pair, m)][:, ds(sub * TCH, TCH)]
                return ptiles[(pair, m)][:, ds(sub * TCH + q * 128, 128)]

            def wg_get(ot, g):
                key = (ot, g)
                if key in wgq:
                    return wgq.pop(key)
                w_ = wp.tile([128, KG, 128], BF16, tag="wg")
                nc.sync.dma_start(w_[:], wt_ext[ot, g])
                return w_

            def wg_prefetch(ot, g):
                w_ = wp.tile([128, KG, 128], BF16, tag="wg")
                nc.sync.dma_start(w_[:], wt_ext[ot, g])
                wgq[(ot, g)] = w_

            def stats_steps(c):
                """Closures: 16 per-ic steps + 1 rows/broadcast step."""
                stp = statps.tile([128, TCH], F32, tag="st", name=f"st{c}")

                def ic_step(ic):
                    def f():
                        xs = xsp.tile([128, TCH], F32, tag="xs")
                        nc.gpsimd.dma_start(
                            xs[:], xt_ext[ds(ic * 128, 128), ds(c * TCH, TCH)]
                        )
                        sq = sqp.tile([128, TCH], BF16, tag="sq")
                        nc.scalar.square(sq[:], xs[:])
                        nc.tensor.matmul(stp[0:1, :], ones_sb[:], xs[:],
                                         start=(ic == 0), stop=(ic == NIC - 1))
                        nc.tensor.matmul(stp[32:33, :], ones_bf[:], sq[:],
                                         start=(ic == 0), stop=(ic == NIC - 1))
                    return f

                def rows_step():
                    mean = rowp.tile([1, TCH], F32, tag="mean", name=f"mean{c}")
                    nc.vector.tensor_scalar_mul(mean[:], stp[0:1, :], 1.0 / I)
                    var = rowp.tile([1, TCH], F32, tag="var", name=f"var{c}")
                    nc.vector.tensor_scalar_mul(var[:], stp[32:33, :], 1.0 / I)
                    row = rowp.tile([1, 2 * TCH], F32, tag="row", name=f"row{c}")
                    rr = row[0:1, 0:TCH]
                    nc.vector.tensor_tensor(rr, mean[:], mean[:], ALU.mult)
                    nc.vector.tensor_sub(var[:], var[:], rr)
                    # std in place of var, istd into the row's first half
                    nc.scalar.activation(var[:], var[:], AF.Sqrt, bias=eps_sb[:])
                    nc.vector.reciprocal(rr, var[:])
                    nc.vector.scalar_tensor_tensor(
                        row[0:1, TCH:2 * TCH], mean[:], -1.0, rr,
                        ALU.mult, ALU.mult,
                    )
                    bc = bcp.tile([128, 2 * TCH], F32, tag="bc", name=f"bc{c}")
                    nc.gpsimd.partition_broadcast(bc[:], row[0:1, :])
                    bc_map[c] = bc

                return [ic_step(ic) for ic in range(NIC)] + [rows_step]

            def gen_chunk(c, deferred=None):
                """Panel generation for chunk c; race matmuls interleaved.
                `deferred` emits the previous chunk's last o-tile drain
                after pair 0 (keeps DVE free to pre-generate pair 0)."""
                bc = bc_map.pop(c)
                iv = bc[:, 0:TCH].unsqueeze(1).broadcast_to((128, 2, TCH))
                nv = bc[:, TCH:2 * TCH].unsqueeze(1).broadcast_to((128, 2, TCH))
                pss = [
                    mmps.tile([128, TCH], F32, tag="ps", name=f"rps{r}_{c}")
                    for r in range(N_RACE)
                ]
                g_next = 0

                def race_mm(g_hi):
                    nonlocal g_next
                    for g in range(g_next, g_hi):
                        for r in range(N_RACE):
                            w_ = wg_get(r, g)
                            for ks in range(KG):
                                sp = g * KG + ks
                                nc.tensor.matmul(
                                    pss[r][:], w_[:, ks, :], rhs(sp),
                                    start=(sp == 0), stop=False,
                                )
                    g_next = g_hi

                for pair in range(NP):
                    xg = xgp.tile([128, 2 * TCH], F32, tag="xg",
                                  name=f"xg_{c}_{pair}")
                    for sub in range(2):
                        ic = 2 * pair + sub
                        nc.gpsimd.dma_start(
                            xg[:, ds(sub * TCH, TCH)],
                            xt_ext[ds(ic * 128, 128), ds(c * TCH, TCH)],
                        )
                    # normalize in place: xn = x*istd + (-mu*istd), per-token
                    nc.vector.tensor_tensor(xg[:], xg[:], iv, ALU.mult)
                    nc.vector.tensor_tensor(xg[:], xg[:], nv, ALU.add)
                    if affine:
                        for sub in range(2):
                            ic = 2 * pair + sub
                            nc.scalar.activation(
                                xg[:, ds(sub * TCH, TCH)], xg[:, ds(sub * TCH, TCH)],
                                AF.Identity,
                                bias=lnb_sb[:, ic:ic + 1], scale=lnw_sb[:, ic:ic + 1],
                            )

                    def pt(m):
                        t_ = panelp.tile([128, 2 * TCH], BF16, tag=f"p{pair}_{m}",
                                         name=f"pan_{c}_{pair}_{m}")
                        ptiles[(pair, m)] = t_
                        return t_

                    th = xgp.tile([128, 2 * TCH], F32, tag="xg",
                                  name=f"th_{c}_{pair}")
                    nc.scalar.activation(th[:], xg[:], AF.Tanh)
                    p0 = panelp.tile([128, 2, TCH], FP8E5, tag=f"p{pair}_0",
                                     name=f"pan_{c}_{pair}_0")
                    ptiles[(pair, 0)] = p0
                    for sub in range(2):
                        nc.scalar.activation(p0[:, sub, :], xg[:, ds(sub * TCH, TCH)],
                                             AF.Silu)
                    sh = shp.tile([128, 2 * TCH], F32, tag="sh")
                    nc.scalar.activation(sh[:], th[:], AF.Sin, scale=math.pi / 2)

                    p1, p2, p3, p4 = pt(1), pt(2), pt(3), pt(4)
                    p5, p6, p7, p8 = pt(5), pt(6), pt(7), pt(8)
                    for sub in range(2):
                        hs = ds(sub * TCH, TCH)
                        shh = sh[:, hs]

                        def scr(tag_i):
                            return scrp.tile([128, TCH], F32, tag="scr",
                                             name=f"scr{tag_i}_{c}_{pair}_{sub}")

                        def lad(tag):
                            return ladp.tile([128, TCH], F32, tag=tag,
                                             name=f"lad_{tag}_{c}_{pair}_{sub}")

                        # u = -2*sh^2 ; c1 = u + 1  (c1 on ACT)
                        u = scr("u")
                        nc.vector.scalar_tensor_tensor(u[:], shh, -2.0, shh,
                                                       ALU.mult, ALU.mult)
                        c1 = lad("c1")
                        nc.vector.tensor_scalar_add(c1[:], u[:], 1.0)
                        sq1 = scr("s1")
                        nc.scalar.square(sq1[:], c1[:])
                        c2 = lad("c2")
                        nc.vector.tensor_scalar(c2[:], sq1[:], 2.0, -1.0,
                                                ALU.mult, ALU.add)
                        u3 = scr("u3")
                        nc.vector.scalar_tensor_tensor(u3[:], c2[:], 2.0, c1[:],
                                                       ALU.mult, ALU.mult)
                        c3 = lad("c3")
                        nc.vector.tensor_sub(c3[:], u3[:], c1[:])
                        sq2 = scr("s2")
                        nc.scalar.square(sq2[:], c2[:])
                        c4 = lad("c4")
                        nc.vector.tensor_scalar(c4[:], sq2[:], 2.0, -1.0,
                                                ALU.mult, ALU.add)
                        # exports m=1..4
                        nc.scalar.copy(p1[:, hs], c1[:])
                        nc.scalar.copy(p2[:, hs], c2[:])
                        nc.scalar.copy(p3[:, hs], c3[:])
                        nc.vector.tensor_copy(p4[:, hs], c4[:])
                        # leaves m=5..8 straight to panel halves
                        u5 = scr("u5")
                        nc.vector.scalar_tensor_tensor(u5[:], c3[:], 2.0, c2[:],
                                                       ALU.mult, ALU.mult)
                        nc.vector.tensor_sub(p5[:, hs], u5[:], c1[:])
                        sq3 = scr("s3")
                        nc.scalar.square(sq3[:], c3[:])
                        nc.scalar.activation(p6[:, hs], sq3[:], AF.Identity,
                                             bias=neg1_sb[:], scale=2.0)
                        u7 = scr("u7")
                        nc.vector.scalar_tensor_tensor(u7[:], c4[:], 2.0, c3[:],
                                                       ALU.mult, ALU.mult)
                        nc.vector.tensor_sub(p7[:, hs], u7[:], c1[:])
                        sq4 = scr("s4")
                        nc.scalar.square(sq4[:], c4[:])
                        nc.scalar.activation(p8[:, hs], sq4[:], AF.Identity,
                                             bias=neg1_sb[:], scale=2.0)

                    if pair == 0 and deferred is not None:
                        deferred()
                    race_mm(2 * (pair + 1))
                race_mm(NG)
                # silu fp8 DoubleRow burst closes each race accumulation
                for r in range(N_RACE):
                    wdr = wdrp.tile([128, NP, 2, 128], FP8E5, tag="wdr",
                                    name=f"rwdr{r}_{c}")
                    nc.sync.dma_start(wdr[:], wt0_ext[r])
                    for pair in range(NP):
                        nc.tensor.matmul(
                            pss[r][:], wdr[:, pair, :, :], ptiles[(pair, 0)][:],
                            start=False, stop=(pair == NP - 1),
                            perf_mode=PM.DoubleRow, skip_group_check=True,
                        )
                for r in range(N_RACE):
                    stg = stgp.tile([128, TCH], BF16, tag="stg",
                                    name=f"rstg{r}_{c}")
                    nc.vector.tensor_copy(stg[:], pss[r][:])
                    nc.gpsimd.dma_start(
                        out_ext[ds(r * 128, 128), ds(c * TCH, TCH)], stg[:]
                    )

            def mm_chunk(c, steps):
                """O-tiles N_RACE..15; silu DR matmuls open each pass;
                next-chunk stats steps injected spread over early
                o-tiles.  Returns a deferred closure for the last
                o-tile's drain (or None)."""
                n_ots = NOT - N_RACE
                deferred = None
                for oi, ot in enumerate(range(N_RACE, NOT)):
                    last = (c == NCH - 1) and (ot == NOT - 1)
                    ps = mmps.tile([128, TCH], F32, tag="ps", name=f"mps{c}_{ot}")
                    wdr = wdrp.tile([128, NP, 2, 128], FP8E5, tag="wdr",
                                    name=f"wdr{c}_{ot}")
                    nc.sync.dma_start(wdr[:], wt0_ext[ot])
                    for pair in range(NP):
                        nc.tensor.matmul(
                            ps[:], wdr[:, pair, :, :], ptiles[(pair, 0)][:],
                            start=(pair == 0), stop=False,
                            perf_mode=PM.DoubleRow, skip_group_check=True,
                        )
                    for g in range(NG):
                        if steps and (g % 3 == 1):
                            steps.pop(0)()
                        w_ = wg_get(ot, g)
                        for ks in range(KG):
                            sp = g * KG + ks
                            if last and sp == NKB - 1:
                                for q in range(4):
                                    nc.tensor.matmul(
                                        ps[:, ds(q * 128, 128)], w_[:, ks, :],
                                        rhs(sp, q), start=False, stop=True,
                                        skip_group_check=True,
                                    )
                            else:
                                nc.tensor.matmul(
                                    ps[:], w_[:, ks, :], rhs(sp),
                                    start=False, stop=(sp == NKB - 1),
                                )

                    def drain(ps=ps, ot=ot):
                        stg = stgp.tile([128, TCH], BF16, tag="stg",
                                        name=f"stg{c}_{ot}")
                        nc.vector.tensor_copy(stg[:], ps[:])
                        nc.gpsimd.dma_start(
                            out_ext[ds(ot * 128, 128), ds(c * TCH, TCH)], stg[:]
                        )

                    if last:
                        # pipelined sliced drain to shorten the kernel tail
                        stg = stgp.tile([128, TCH], BF16, tag="stg",
                                        name=f"stg{c}_{ot}")
                        for q in range(4):
                            sl = ds(q * 128, 128)
                            nc.vector.tensor_copy(stg[:, sl], ps[:, sl])
                            nc.gpsimd.dma_start(
                                out_ext[ds(ot * 128, 128),
                                        ds(c * TCH + q * 128, 128)],
                                stg[:, sl],
                            )
                    elif oi == n_ots - 1 and c + 1 < NCH:
                        deferred = drain
                    else:
                        drain()
                while steps:
                    steps.pop(0)()
                return deferred

            # --- program ---
            for g in range(2):
                for r in range(N_RACE):
                    if len(wgq) < 5:
                        wg_prefetch(r, g)
            for f in stats_steps(0):
                f()
            deferred = None
            for c in range(NCH):
                gen_chunk(c, deferred=deferred)
                steps = stats_steps(c + 1) if c + 1 < NCH else []
                deferred = mm_chunk(c, steps)

    nc.compile()
    return nc


def prep_weights(base_weight, spline_weight):
    """Host-side weight prep.
    wt  (bf16 cos part): k-step sp = pair*16 + (m-1)*2 + sub, tiled
        [ot, g, k_in, ks, o_in] for [128, KG, 128] DMAs.
    wt0 (fp8e5 silu part, DoubleRow): [ot, k_in, pair, sub, o_in]."""
    w = np.empty((G, I, O), np.float32)
    for g in range(G):
        w[g] = spline_weight[:, :, g].T       # [i, o]
    w = w.reshape(G, NP, 2, 128, O).transpose(1, 0, 2, 3, 4)  # [pair, m-1, sub, 128, o]
    w = w.reshape(NKB * 128, O)
    w = w.reshape(NG, KG, 128, NOT, 128).transpose(3, 0, 2, 1, 4)
    wt = np.ascontiguousarray(w.astype(ml_dtypes.bfloat16))
    wb = base_weight.T.reshape(NP, 2, 128, NOT, 128)  # [pair, sub, kin, ot, oin]
    wb = wb.transpose(3, 2, 0, 1, 4)                  # [ot, kin, pair, sub, oin]
    wt0 = np.ascontiguousarray(wb.astype(ml_dtypes.float8_e5m2))
    return wt, wt0


_NC_CACHE = {}


def _get_nc(affine=False):
    if affine not in _NC_CACHE:
        _NC_CACHE[affine] = build_nc(affine=affine)
    return _NC_CACHE[affine]


def kernel(x, ln_weight, ln_bias, base_weight, spline_weight):
    x = np.asarray(x, np.float32)
    ln_weight = np.asarray(ln_weight, np.float32)
    ln_bias = np.asarray(ln_bias, np.float32)
    affine = not (np.all(ln_weight == 1.0) and np.all(ln_bias == 0.0))
    wt, wt0 = prep_weights(np.asarray(base_weight, np.float32),
                           np.asarray(spline_weight, np.float32))
    nc = _get_nc(affine)
    in_maps = [
        {
            "xt": np.ascontiguousarray(x[b].T),
            "lnw": ln_weight,
            "lnb": ln_bias,
            "wt": wt,
            "wt0": wt0,
        }
        for b in range(B)
    ]
    res = run_bass_kernel_spmd(nc, in_maps, core_ids=list(range(B)))
    out = np.stack([res.results[b]["out"].astype(np.float32).T for b in range(B)])
    return np.ascontiguousarray(out)
